# revision 65
# baseline (speedup 1.0000x reference)
"""Trainium2 Bass kernel for nn_AttentionLayer (B=4, S=2048, H=16, DH=64).

Sharding: 8 cores = 4 batches x 2 head-halves. Core c handles batch c//2,
heads (c%2)*8 .. (c%2)*8+8 (i.e. 512 of the 1024 QKV columns).

Per-core device program (SPMD, same program on all cores, different inputs):
  inputs (pre-laid-out on host, x/W in bf16 to halve DMA):
    xT  [1024, 2048] bf16 = x[b].T       (contraction dim on partitions)
    wkq [4, 128, 8, 256] bf16            (Wk|Wq packed per head-pair in the
                                          SBUF tile layout, 4KB rows)
    wv2 [4, 128, 8, 128] bf16            (Wv likewise, per head-pair)
    bq/bk/bv [512] f32
  output:
    out [2048, 512] bf16 = attention ctx, seq-major (q rows, head*64+dh
    cols); host only reshapes and casts to f32.

Design (vs the f32r baseline, which ran PV as V_aug.T @ E at N=512;
331.8us modeled -> 288.5us here; hw-verified 288462ns). ACT (exp) is the bottleneck engine at
its floor: 256 ops x (1024*0.833 + 185 access)ns ~ 267us busy; everything
else is arranged to keep it fed.
  - All inputs bf16 (rel err ~3.7e-3 vs the f32 reference, budget 2e-2):
    x streamed ONCE and kept resident (32KB/part; the baseline re-streamed
    it per pair), one full-rate dma_start for Wk+Wq per pair, x chunks in
    quarter/half sub-issues so projection chains start on partial data
    (HWDGE issue overhead is ~0.63us per dma_start); Wv loaded after x.
  - qt/kt projected per head-pair into bf16 [128, S]; scores
    [128 kpos, q] via N=512 matmuls (full rate), exp on ACT PSUM->SBUF
    with bf16 output, software-pipelined two score-blocks ahead.
  - PV transposed: ctx[q, dh] += E[:, q128].T @ V_aug[kb], with
    V_aug = [V | ones] bf16 [128, 65]; N=65 at bf16 full rate (f32r is
    4x-penalized at N<256). Column 64 accumulates the softmax denominator
    for free. Total PE ~721k -> ~592k cycles (~250us busy).
  - V chains are emitted PER HEAD-PAIR (rhs = 128-col slice of Wv, same
    total cycles) inside that pair's filler window, with that pair's Wv
    slice loaded there too: if pair 0's window carries all of V, the exp
    pipeline stalls ~18us at the pair-0/pair-1 boundary draining the
    deferred-PV backlog.
  - PSUM zero-region rule (start=True lazily zeroes the whole 2KB bank):
    each unit's ctx lives in TWO banks of [128, 4, 65] with exactly one
    accumulation group per bank. PSUM: scores 2x2 banks + ctx 2x1 +
    qkv 2 = 8. epool holds 48 E tiles so exp never waits on PV/vt.
  - Normalize on DVE: bulk-copy both ctx banks PSUM->SBUF (frees them for
    the next unit's PV), reciprocal of the denominator column, then
    per-subtile (ctx * 1/den) + bv (folded post-normalize, sum(p)=1),
    seq-major bf16 DMA per 4-subtile half.
  - 8 dummy warm-up matmuls ramp the PE p-state during the first DMAs; a
    dummy exp pulls the ACT table load to t=0.
"""

import numpy as np

B, S, H, DH = 4, 2048, 16, 64
D = H * DH  # 1024
NCORES = 8
COLS = 512  # qkv columns per core (8 heads)
NPAIR = 4  # head pairs per core
NKB = S // 128  # 16 k-blocks
QG = 1024  # q-group width
NQG = S // QG  # 2
XC = 512  # x streaming chunk (seq cols)
NXC = S // XC  # 4
INV_SQRT_DH = 1.0 / 8.0

_CACHE = {}


def _build():
    import concourse.mybir as mybir
    import concourse.tile as tile
    from concourse import bacc

    f32 = mybir.dt.float32
    f32r = mybir.dt.float32r
    bf16 = mybir.dt.bfloat16
    Exp = mybir.ActivationFunctionType.Exp
    Alu = mybir.AluOpType

    nc = bacc.Bacc(
        "TRN2",
        target_bir_lowering=False,
        debug=False,
        enable_asserts=False,
        num_devices=NCORES,
    )

    xT_d = nc.dram_tensor("xT", [D, S], bf16, kind="ExternalInput").ap()
    # Wk|Wq host-pre-rearranged+packed to SBUF tile layout [128, 8, 256]
    # (wk in cols 0:128, wq in 128:256): 4KB contiguous rows, one full-rate
    # dma_start per pair for both weights
    wkq_d = nc.dram_tensor(
        "wkq", [NPAIR, 128, 8, 256], bf16, kind="ExternalInput"
    ).ap()
    # Wv host-pre-rearranged per head-pair [128, 8, 128] (2KB rows): each
    # pair's slice loads at full rate (0.73us) inside that pair's window,
    # so pair 0's V chains are not gated on a monolithic 2.9us Wv load
    wv2_d = nc.dram_tensor(
        "wv2", [NPAIR, 128, 8, 128], bf16, kind="ExternalInput"
    ).ap()
    bq_d = nc.dram_tensor("bq", [COLS], f32, kind="ExternalInput").ap()
    bk_d = nc.dram_tensor("bk", [COLS], f32, kind="ExternalInput").ap()
    bv_d = nc.dram_tensor("bv", [COLS], f32, kind="ExternalInput").ap()
    out_d = nc.dram_tensor("out", [S, COLS], bf16, kind="ExternalOutput").ap()

    with tile.TileContext(nc) as tc:
        with (
            tc.tile_pool(name="consts", bufs=1) as consts,
            tc.tile_pool(name="vpool", bufs=1) as vpool,
            tc.tile_pool(name="wvpool", bufs=1) as wvpool,
            tc.tile_pool(name="wqk", bufs=2) as wqk,
            tc.tile_pool(name="xpool", bufs=1) as xpool,
            tc.tile_pool(name="qkt", bufs=2) as qkt,
            tc.tile_pool(name="epool", bufs=50) as epool,
            tc.tile_pool(name="opool", bufs=3) as opool,
            tc.tile_pool(name="psum", bufs=1, space="PSUM") as psum,
        ):
            # ---- constants, ACT table preload ----
            bq_t = consts.tile([128, NPAIR], f32)
            bk_t = consts.tile([128, NPAIR], f32)
            bv_s = consts.tile([1, COLS], f32)
            bvb = consts.tile([128, 8, DH], f32)  # bv broadcast per head
            nc.gpsimd.dma_start(out=bq_t, in_=bq_d.rearrange("(m p) -> p m", p=128))
            nc.gpsimd.dma_start(out=bk_t, in_=bk_d.rearrange("(m p) -> p m", p=128))
            nc.gpsimd.dma_start(out=bv_s, in_=bv_d[None, :])
            for h in range(8):
                nc.gpsimd.partition_broadcast(
                    bvb[:, h, :], bv_s[:, h * DH : (h + 1) * DH]
                )
            warm = consts.tile([1, 1], f32)
            nc.vector.memset(warm, 0.0)
            nc.scalar.activation(warm, warm, Exp)  # pull ACT table load early
            # ramp the PE p-state during the initial DMA wait: ~4us of dummy
            # matmuls so the first real projection runs at full clock
            wzero = consts.tile([128, 512], bf16)
            nc.vector.memset(wzero, 0.0)
            wps = psum.tile([128, 512], f32, tag="p1", bufs=2, name="wps")
            for i in range(8):
                nc.tensor.matmul(
                    wps,
                    lhsT=wzero[:, 0:128],
                    rhs=wzero,
                    start=(i == 0),
                    stop=(i == 7),
                )

            # V_aug, bf16: vt[i][:, h, 0:64] = V chunk, col 64 = ones (denom)
            vt = [vpool.tile([128, 8, 65], bf16, name=f"vt{i}") for i in range(NKB)]
            for i in range(NKB):
                nc.vector.memset(vt[i][:, :, 64:65], 1.0)

            wv = wvpool.tile([128, 8, COLS], bf16, name="wv")

            def load_wv_pair(m):
                nc.sync.dma_start(
                    out=wv[:, :, m * 128 : (m + 1) * 128], in_=wv2_d[m]
                )

            def load_x_chunk(c, fine=False):
                # x[b] in bf16 is only 32KB/partition: stream it ONCE and keep
                # it resident; every head-pair's projections reuse it.
                # 2 half-issues (4 quarter-issues for the startup-critical
                # chunks) so projection chains start on partial data; 8 per-j
                # dma_starts would serialize on HWDGE issue overhead.
                xt = xpool.tile([128, 8, XC], bf16, name=f"xt{c}", tag=f"xt{c}")
                n = 4 if fine else 2
                for hh in range(n):
                    nc.sync.dma_start(
                        out=xt[:, hh * 8 // n : (hh + 1) * 8 // n, :],
                        in_=xT_d[
                            hh * 1024 // n : (hh + 1) * 1024 // n,
                            c * XC : (c + 1) * XC,
                        ].rearrange("(j p) c -> p j c", p=128),
                    )
                return xt

            def load_w_pair(m, defer_dma=False):
                wkqm = wqk.tile([128, 8, 256], bf16, name="wkqm", tag="wkqm")
                if not defer_dma:
                    nc.sync.dma_start(out=wkqm, in_=wkq_d[m])
                return wkqm[:, :, 128:256], wkqm[:, :, 0:128], wkqm

            def qk_chunk(m, c, xt, wqm, wkm, qt, kt, projs=("k", "q"), tag="p1"):
                pairs = {"q": (wqm, bq_t, qt), "k": (wkm, bk_t, kt)}
                for w, bias, dst in (pairs[p] for p in projs):
                    ps = psum.tile([128, 512], f32, tag=tag, bufs=2)
                    for j in range(8):
                        nc.tensor.matmul(
                            ps,
                            lhsT=w[:, j, :],
                            rhs=xt[:, j, :],
                            start=(j == 0),
                            stop=(j == 7),
                        )
                    nc.vector.tensor_scalar_add(
                        dst[:, c * XC : (c + 1) * XC], ps, bias[:, m : m + 1]
                    )

            def v_chunk(c, xt, mm):
                # per-PAIR V slice (rhs N=128, same total PE cycles as the
                # 8-head chain): pair m's V is emitted in pair m's own filler
                # window, so pair 0's window is not flooded by all of V
                for i in range(XC // 128):
                    it = c * (XC // 128) + i
                    ps = psum.tile([128, 128], f32, tag="p1", bufs=2)
                    for j in range(8):
                        nc.tensor.matmul(
                            ps,
                            lhsT=xt[:, j, i * 128 : (i + 1) * 128],
                            rhs=wv[:, j, mm * 128 : (mm + 1) * 128],
                            start=(j == 0),
                            stop=(j == 7),
                        )
                    # bv folded in post-normalize (sum p = 1): plain cast
                    # copy (DVE: GPSIMD cannot access PSUM on TRN2)
                    nc.vector.tensor_copy(
                        vt[it][:, 2 * mm : 2 * mm + 2, 0:64],
                        ps.rearrange("p (h d) -> p h d", h=2),
                    )

            def emit_attention_unit(m, q0, p0, head, qt, kt, qw=QG):
                nsub = qw // 128
                nhalf = (nsub + 3) // 4
                ctx = [
                    psum.tile(
                        [128, 4, 65], f32, tag=f"ctx{i}", bufs=1, name=f"ctx{i}"
                    )
                    for i in range(nhalf)
                ]

                def scores(kb):
                    sc = psum.tile([128, QG], f32, tag="sc", bufs=2, name="sc")
                    for qq in range(qw // 512):
                        nc.tensor.matmul(
                            sc[:, qq * 512 : (qq + 1) * 512],
                            lhsT=kt[p0 : p0 + 64, kb * 128 : (kb + 1) * 128],
                            rhs=qt[
                                p0 : p0 + 64,
                                q0 + qq * 512 : q0 + (qq + 1) * 512,
                            ],
                            start=True,
                            stop=True,
                        )
                    return sc

                # software-pipelined emission, two scores ahead (see baseline)
                scs = [scores(0), scores(1)]
                for kb in range(NKB):
                    ee = epool.tile([128, QG], bf16, tag="e", name="ee")
                    nc.scalar.activation(
                        ee[:, 0:qw], scs[kb % 2][:, 0:qw], Exp, scale=INV_SQRT_DH
                    )
                    if kb < NKB - 2:
                        scs[kb % 2] = scores(kb + 2)
                    for s in range(nsub):
                        # one accumulation group per PSUM bank (zero region):
                        # start on the bank's first write, stop on its last
                        nc.tensor.matmul(
                            ctx[s // 4][:, s % 4, :],
                            lhsT=ee[:, s * 128 : (s + 1) * 128],
                            rhs=vt[kb][:, head, :],
                            start=(kb == 0 and s % 4 == 0),
                            stop=(kb == NKB - 1 and s % 4 == 3),
                        )
                # normalize: bulk-copy BOTH ctx banks PSUM->SBUF first (frees
                # them for the next unit's PV asap), then recip + per-subtile
                # (ctx * 1/den) + bv from SBUF, seq-major DMA per half
                css = []
                for i in range(nhalf):
                    cs = opool.tile([128, 4, 65], f32, tag="c", name="cs")
                    nc.vector.tensor_copy(cs, ctx[i])
                    css.append(cs)
                for i, cs in enumerate(css):
                    rr = opool.tile([128, 4, 1], f32, tag="r", name="rr")
                    nc.vector.reciprocal(rr, cs[:, :, 64:65])
                    ob = opool.tile([128, 4, DH], bf16, tag="o", name="ob")
                    for s in range(4):
                        nc.vector.scalar_tensor_tensor(
                            ob[:, s, :],
                            cs[:, s, 0:64],
                            rr[:, s, :],
                            bvb[:, head, :],
                            op0=Alu.mult,
                            op1=Alu.add,
                        )
                    nc.sync.dma_start(
                        out=out_d[
                            q0 + i * 512 : q0 + (i + 1) * 512,
                            head * DH : (head + 1) * DH,
                        ].rearrange("(s p) d -> p s d", p=128),
                        in_=ob,
                    )

            # ---- banded priorities: attention preferred, QKV/V fill gaps ----
            from contextlib import contextmanager

            base = tc.cur_priority + 50
            att_cur = [base]
            fill_cur = [base + 6000]

            @contextmanager
            def band(cursor):
                off = tc.cur_priority - cursor[0]
                with tc.high_priority(offset=off):
                    yield
                    cursor[0] = tc.cur_priority

            # ---- per pair: QKV (filler band) then attention (att band) ----
            xts = {}
            for m in range(NPAIR):
                with band(fill_cur):
                    wqm, wkm, wkqm = load_w_pair(m, defer_dma=(m == 0))
                    qt = qkt.tile([128, S], bf16, name=f"qt{m}", tag="qt")
                    kt = qkt.tile([128, S], bf16, name=f"kt{m}", tag="kt")
                    if m == 0:
                        # all QK chunks first (kt feeds every unit's scores;
                        # exp stalls if kt c2/c3 are late), V strictly after
                        nc.sync.dma_start(out=wkqm, in_=wkq_d[m])
                        xts[0] = load_x_chunk(0, fine=True)
                        xts[1] = load_x_chunk(1, fine=True)
                        load_wv_pair(0)
                        xts[2] = load_x_chunk(2, fine=True)
                        xts[3] = load_x_chunk(3, fine=True)
                        # exp(0) needs kt chunk 0 + qt chunks 0 AND 1, but
                        # kt chunk 1 only by exp(4): project q of chunk 1
                        # before k of chunk 1 to pull the first exp earlier.
                        # q0 goes through an idle scores-pool slot so the
                        # three startup chains have conflict-free PSUM slots
                        qk_chunk(m, 0, xts[0], wqm, wkm, qt, kt, projs=("k",))
                        qk_chunk(m, 0, xts[0], wqm, wkm, qt, kt, projs=("q",), tag="sc")
                        qk_chunk(m, 1, xts[1], wqm, wkm, qt, kt, projs=("q",))
                        qk_chunk(m, 1, xts[1], wqm, wkm, qt, kt, projs=("k",))
                        qk_chunk(m, 2, xts[2], wqm, wkm, qt, kt)
                        qk_chunk(m, 3, xts[3], wqm, wkm, qt, kt)
                        for c in range(NXC):
                            v_chunk(c, xts[c], m)
                    else:
                        load_wv_pair(m)
                        for c in range(NXC):
                            qk_chunk(
                                m, c, xts[c], wqm, wkm, qt, kt,
                                projs=("k", "q") if c < 2 else ("k",),
                            )
                        for c in range(NXC):
                            v_chunk(c, xts[c], m)

                # ---- attention units (Q c2/c3 deferred after qg0) ----
                for qg in range(NQG):
                    q0 = qg * QG
                    for h in range(2):
                        head = 2 * m + h
                        p0 = h * 64
                        with band(att_cur):
                            emit_attention_unit(m, q0, p0, head, qt, kt)
                    if qg == 0 and m > 0:
                        with band(fill_cur):
                            for c in (2, 3):
                                qk_chunk(m, c, xts[c], wqm, wkm, qt, kt, projs=("q",))

    nc.compile()
    return nc


def _get_nc():
    if "nc" not in _CACHE:
        _CACHE["nc"] = _build()
    return _CACHE["nc"]


def _w_slab(wk, wq):
    # [D, 512] x2 -> [NPAIR, 128, 8, 256]: per pair m, wk cols m*128:(m+1)*128
    # at [..., 0:128] and wq's at [..., 128:256], rows (j*128+p) -> [p, j, c]
    # (the SBUF tile layout, 4KB contiguous rows)
    import ml_dtypes

    out = np.empty((NPAIR, 128, 8, 256), ml_dtypes.bfloat16)
    for m in range(NPAIR):
        sl = slice(m * 128, (m + 1) * 128)
        out[m, :, :, 0:128] = (
            wk[:, sl].reshape(8, 128, 128).transpose(1, 0, 2)
        ).astype(ml_dtypes.bfloat16)
        out[m, :, :, 128:256] = (
            wq[:, sl].reshape(8, 128, 128).transpose(1, 0, 2)
        ).astype(ml_dtypes.bfloat16)
    return np.ascontiguousarray(out)


def _wv_slab(w):
    # [D, 512] -> [NPAIR, 128, 8, 128]: pair m's cols in the SBUF tile layout
    import ml_dtypes

    out = np.empty((NPAIR, 128, 8, 128), ml_dtypes.bfloat16)
    for m in range(NPAIR):
        out[m] = (
            w[:, m * 128 : (m + 1) * 128].reshape(8, 128, 128).transpose(1, 0, 2)
        ).astype(ml_dtypes.bfloat16)
    return np.ascontiguousarray(out)


def _in_maps(x, Wq, bq, Wk, bk, Wv, bv):
    import ml_dtypes

    bf = ml_dtypes.bfloat16
    maps = []
    for c in range(NCORES):
        b, hh = c // 2, c % 2
        cs = slice(hh * COLS, (hh + 1) * COLS)
        maps.append(
            {
                "xT": np.ascontiguousarray(np.asarray(x)[b].T).astype(bf),
                "wkq": _w_slab(np.asarray(Wk)[:, cs], np.asarray(Wq)[:, cs]),
                "wv2": _wv_slab(np.asarray(Wv)[:, cs]),
                "bq": np.ascontiguousarray(np.asarray(bq)[cs]),
                "bk": np.ascontiguousarray(np.asarray(bk)[cs]),
                "bv": np.ascontiguousarray(np.asarray(bv)[cs]),
            }
        )
    return maps


def _run(inputs, trace=False):
    from concourse import bass_utils

    nc = _get_nc()
    res = bass_utils.run_bass_kernel_spmd(
        nc,
        _in_maps(**inputs),
        core_ids=list(range(NCORES)),
        trace=trace,
    )
    out = np.empty((B, S, D), np.float32)
    for c in range(NCORES):
        b, hh = c // 2, c % 2
        out[b, :, hh * COLS : (hh + 1) * COLS] = res.results[c]["out"].astype(
            np.float32
        )
    return out, res


def kernel(**inputs):
    out, _ = _run(inputs, trace=False)
    return out


if __name__ == "__main__":
    _get_nc()
    print("build ok")
